# revision 18
# baseline (speedup 1.0000x reference)
"""Trainium2 Bass kernel for nn_CIP_44392781971895.

Math: the reference computes, per (b, m, t),
    joint[b,m,t] = min( prod_{s,n} pdf(z[b,m,s,n]; mean_T[t,s,n], var[t,s,n])
                        * 4.13273 * std_T0[n], 1e20 )
then num_y = einsum('bmt,tsy', joint, y_true_T), num = sum_t joint,
probs = max(num_y,1e-20)/max(num,1e-20), mean over m, clip to [0,1].

The product over the 512 (s,n) pairs is computed in log space, which
collapses to a matmul over the flattened sn axis:

    logit[t,bm] = cval + sum_sn[ A2*z - 0.5*e*z^2 - 0.5*lvT - 0.5*q ]
      e  = exp(-log_var_T)   (= 1/var; the reference's 1e-20 variance
           floor binds only for log_var_T < -46, far outside the input
           distribution, so it is not applied)
      A2 = e * mean_T,  q = e * mean_T^2
      cval = S*N*(log 4.13273 - 0.5 log 2pi) + (S/2) * sum_n log_var_T[0,0,:]
    joint = exp(logit)
(The reference's min(.,1e20) clamp binds only for logit > 46; the log-joints
for this problem sit far below the fp32-exp underflow threshold, with 380+
log-units of margin, so the clamp is inert and omitted.)

All tables are laid out sn-major (pre-transposed on the host), so the
contraction runs directly over the partition axis with NO on-device
transposes: 4 chunks of 128 sn-rows, t in the free dimension.  The per-t
constants sum(-0.5*lvT) and sum(-0.5*q) are folded into the same PSUM
accumulation as extra matmul chunks against a constant -0.5 tile, and cval
enters through the exp bias column.

Sharding: the T=2000 prototype axis is split across the 8 cores (250 each).
The shard is covered by two overlapping 128-wide t-tiles (0:128, 122:250);
the 6 duplicated prototypes have their y/ones columns zeroed in the second
tile so stage-2 counts them once.  Both tiles' logits live in one PSUM bank
([128, 128]) and share a single exp; both stage-2 matmuls accumulate into
one PSUM bank, DMA'd out as a single (64, 161) partial that the host sums
across cores and finishes (divide / mean over m / clip).

Precision: tables, z samples, stage-1 operands, joints, and the stage-2
operands are bf16 (fp32 PSUM accumulation everywhere).  The bf16-induced
logit error (a few units) is inconsequential against the 380+ log-unit
underflow margin, and y_true in bf16 is well inside the 2e-2 tolerance.

Raw Bass (explicit engine blocks + single-event semaphores); input DMAs
are spread across the SP and Pool queues so they issue concurrently, an
early dummy matmul starts the PE clock-ramp, and the output copy + DMA
ride the Activation queue back-to-back.
"""

from contextlib import ExitStack

import ml_dtypes
import numpy as np

import concourse.bass as bass
import concourse.mybir as mybir

NCORES = 8
B, S, N = 32, 16, 32
T, M, Y = 2000, 2, 10
SN = S * N            # 512  (contraction length per prototype)
BM = B * M            # 64   (flattened batch*samples, column index m*B + b)
TSH = T // NCORES     # 250  (prototypes per core)
SY = S * Y            # 160
F32 = mybir.dt.float32
BF16 = mybir.dt.bfloat16
NPBF = ml_dtypes.bfloat16

KONST = float(SN * (np.log(np.float64(4.13273)) - 0.5 * np.log(2.0 * np.pi)))

T_TILES = [0, TSH - 128]   # start t of the two (overlapping) 128-wide tiles
ZW = 192                   # per-chunk zint row: lv|mean|eps
YW = SY + 1                # per-tile ytb16 row: y(160) | ones


def build_program() -> bass.Bass:
    nc = bass.Bass()
    AF = mybir.ActivationFunctionType
    OP = mybir.AluOpType

    # Packed inputs (built host-side in make_in_maps), tables sn-chunk-major:
    #   lvt:  (128, 1000) bf16  lvt[p, c*250+j] = log_var_T[shard j, sn c*128+p]
    #   mtt:  (128, 1000) bf16  same layout for mean_T
    #   m2t:  (128, 1000) bf16  same layout for mean_T^2
    #   zint: (128, 768)  bf16  chunk c cols [c*192,(c+1)*192) =
    #         [lv.T dup(64) | mean.T dup(64) | eps.T(64)] for sn c*128+p
    #   ytb:  (128, 322)  bf16  tile ti cols [ti*161,...): [y(160) | 1]
    #   ycv:  (128, 1)    f32   cval (exp bias column)
    lvt_d = nc.dram_tensor("lvt", [128, 4 * TSH], BF16, kind="ExternalInput")
    mtt_d = nc.dram_tensor("mtt", [128, 4 * TSH], BF16, kind="ExternalInput")
    m2t_d = nc.dram_tensor("m2t", [128, 4 * TSH], BF16, kind="ExternalInput")
    zint_d = nc.dram_tensor("zint", [128, 4 * ZW], BF16, kind="ExternalInput")
    ytb_d = nc.dram_tensor("ytb", [128, 2 * YW], BF16, kind="ExternalInput")
    ycv_d = nc.dram_tensor("ycv", [128, 1], F32, kind="ExternalInput")
    part_d = nc.dram_tensor("partial", [BM, SY + 1], F32, kind="ExternalOutput")

    es = ExitStack()
    with es:
        sb = lambda name, shape, dt=BF16: es.enter_context(nc.sbuf_tensor(name, shape, dt))
        ps = lambda name, shape, dt: es.enter_context(nc.psum_tensor(name, shape, dt))

        lvt = sb("s_lvt", [128, 4 * TSH])
        mtt = sb("s_mtt", [128, 4 * TSH])
        m2t = sb("s_m2t", [128, 4 * TSH])
        zint = sb("s_zint", [128, 4 * ZW])
        ytb = sb("s_ytb", [128, 2 * YW])
        ycv = sb("s_ycv", [128, 1], F32)
        eT = sb("s_eT", [128, 4 * TSH])
        A2T = sb("s_A2T", [128, 4 * TSH])
        qT = sb("s_qT", [128, 4 * TSH])
        X = sb("s_X", [128, 8 * BM])       # [z chunks 0..3 | -0.5 z^2 chunks]
        std4 = sb("s_std4", [128, 4 * BM])
        joint = sb("s_joint", [128, 2 * BM])
        neg64 = sb("s_neg64", [128, BM])   # bf16 -0.5 tile (C-chunk rhs)
        out_sb = sb("s_out", [BM, SY + 1], F32)
        bz16 = sb("s_bz16", [128, 1])      # bf16 zeros (exp bias)
        warm = sb("s_warm", [1, 1])

        pl = ps("p_l", [128, 2 * BM], F32)   # logits, tile ti in cols ti*64..
        po = ps("p_o", [BM, SY + 1], F32)
        pdum = ps("p_dum", [BM, 1], F32)

        sem = lambda name: es.enter_context(nc.semaphore(name))
        t_lv, t_mt, t_m2, t_z = (sem(n) for n in ("t_lv", "t_mt", "t_m2", "t_z"))
        t_y16, t_yc = sem("t_y16"), sem("t_yc")
        s_bias, s_ng, s_std, s_x = sem("s_bias"), sem("s_ng"), sem("s_std"), sem("s_x")
        s_ea, s_eb = sem("s_ea"), sem("s_eb")
        s_a2a, s_a2b = sem("s_a2a"), sem("s_a2b")
        s_qa, s_qb = sem("s_qa"), sem("s_qb")
        s_mm = [sem("s_mm0"), sem("s_mm1")]
        s_j, s_s2, s_oc, s_od = sem("s_j"), sem("s_s2"), sem("s_oc"), sem("s_od")
        s_oc2 = sem("s_oc2")

        zview = zint[:].rearrange("p (c k) -> p c k", k=ZW)
        lv4 = zview[:, :, 0:BM]
        mean4 = zview[:, :, BM:2 * BM]
        eps4 = zview[:, :, 2 * BM:3 * BM]
        std4v = std4[:].rearrange("p (c k) -> p c k", k=BM)
        X0v = X[:, 0:4 * BM].rearrange("p (c k) -> p c k", k=BM)

        def tsl(tbl, c, ti):
            t0 = T_TILES[ti]
            return tbl[:, c * TSH + t0: c * TSH + t0 + 128]

        H = 2 * TSH   # column split of the sn-major tables (chunks 0-1 | 2-3)

        with nc.Block() as block:

            @block.sync
            def _(sync):
                sync.dma_start(zint[:], zint_d[:]).then_inc(t_z, 16)
                sync.dma_start(lvt[:], lvt_d[:]).then_inc(t_lv, 16)
                sync.dma_start(mtt[:], mtt_d[:]).then_inc(t_mt, 16)
                sync.dma_start(ycv[:], ycv_d[:]).then_inc(t_yc, 16)

            @block.gpsimd
            def _(gp):
                gp.memset(bz16[:], 0.0).then_inc(s_bias, 1)
                gp.dma_start(m2t[:], m2t_d[:]).then_inc(t_m2, 16)
                gp.dma_start(ytb[:], ytb_d[:]).then_inc(t_y16, 16)
                gp.wait_ge(s_ea, 1)
                gp.wait_ge(t_mt, 16)
                gp.tensor_mul(A2T[:, 0:H], eT[:, 0:H], mtt[:, 0:H]).then_inc(s_a2a, 1)
                gp.wait_ge(s_eb, 1)
                gp.tensor_mul(A2T[:, H:2 * H], eT[:, H:2 * H],
                              mtt[:, H:2 * H]).then_inc(s_a2b, 1)

            @block.vector
            def _(vector):
                vector.memset(neg64[:], -0.5).then_inc(s_ng, 1)
                vector.wait_ge(t_z, 16)
                vector.wait_ge(s_std, 1)
                vector.tensor_mul(X0v, eps4, std4v)
                vector.drain()
                vector.tensor_add(X0v, X0v, mean4)
                vector.drain()
                vector.scalar_tensor_tensor(
                    X[:, 4 * BM:8 * BM], X[:, 0:4 * BM], -0.5, X[:, 0:4 * BM],
                    op0=OP.mult, op1=OP.mult).then_inc(s_x, 1)
                vector.wait_ge(s_ea, 1)
                vector.wait_ge(t_m2, 16)
                vector.tensor_mul(qT[:, 0:H], eT[:, 0:H],
                                  m2t[:, 0:H]).then_inc(s_qa, 1)
                vector.wait_ge(s_eb, 1)
                vector.tensor_mul(qT[:, H:2 * H], eT[:, H:2 * H],
                                  m2t[:, H:2 * H]).then_inc(s_qb, 1)
                vector.wait_ge(s_s2, 1)
                vector.tensor_copy(out_sb[:, 80:SY + 1],
                                   po[:, 80:SY + 1]).then_inc(s_oc2, 1)

            @block.scalar
            def _(scalar):
                # prewarm the ACT Exp table while DMAs are in flight
                scalar.wait_ge(s_bias, 1)
                scalar.activation(warm[:], bz16[0:1, :], AF.Exp,
                                  bias=bz16[0:1, :])
                scalar.wait_ge(t_z, 16)
                scalar.activation(std4[:], lv4, AF.Exp, bias=bz16[:, :],
                                  scale=0.5).then_inc(s_std, 1)
                scalar.wait_ge(t_lv, 16)
                scalar.activation(eT[:, 0:H], lvt[:, 0:H], AF.Exp,
                                  bias=bz16[:, :], scale=-1.0).then_inc(s_ea, 1)
                scalar.activation(eT[:, H:2 * H], lvt[:, H:2 * H], AF.Exp,
                                  bias=bz16[:, :], scale=-1.0).then_inc(s_eb, 1)
                # joints: one exp over both tiles' logits, cval via bias
                scalar.wait_ge(s_mm[0], 1)
                scalar.wait_ge(t_yc, 16)
                scalar.activation(joint[:], pl[:, :], AF.Exp,
                                  bias=ycv[:, 0:1]).then_inc(s_j, 1)
                # output: PSUM -> SBUF (split with DVE) -> DRAM on this queue
                scalar.wait_ge(s_s2, 1)
                scalar.copy(out_sb[:, 0:80], po[:, 0:80]).then_inc(s_oc, 1)
                scalar.wait_ge(s_oc, 1)
                scalar.wait_ge(s_oc2, 1)
                scalar.dma_start(part_d[:], out_sb[:]).then_inc(s_od, 16)

            @block.tensor
            def _(tensor):
                # dummy matmul: start the PE p-state ramp clock early
                tensor.wait_ge(s_ng, 1)
                nc.tensor.matmul(pdum[:], neg64[:, 0:BM], neg64[:, 0:1],
                                 start=True, stop=True)
                # stage-1: 16 chunk-matmuls per 128-wide t-tile accumulating
                #   logit = z@A2 - 0.5 z^2 @ e - 0.5*(lvT + q) @ 1
                # into pl cols ti*64..; ordered by operand readiness.
                def grp(tbl, rhs_of, cs, start=False, stop=False, inc=None):
                    for ti in range(2):
                        for c in cs:
                            ins = nc.tensor.matmul(
                                pl[:, ti * BM:(ti + 1) * BM], tsl(tbl, c, ti),
                                rhs_of(c),
                                start=start and ti == 0 and c == cs[0],
                                stop=stop and ti == 1 and c == cs[-1])
                            if inc is not None and ti == 1 and c == cs[-1]:
                                ins.then_inc(inc, 1)

                Xz = lambda c: X[:, c * BM:(c + 1) * BM]
                X2 = lambda c: X[:, (4 + c) * BM:(5 + c) * BM]
                Ng = lambda c: neg64[:, 0:BM]

                tensor.wait_ge(t_lv, 16)
                grp(lvt, Ng, [0, 1, 2, 3], start=True)
                tensor.wait_ge(s_ea, 1)
                tensor.wait_ge(s_x, 1)
                grp(eT, X2, [0, 1])
                tensor.wait_ge(s_a2a, 1)
                grp(A2T, Xz, [0, 1])
                tensor.wait_ge(s_eb, 1)
                grp(eT, X2, [2, 3])
                tensor.wait_ge(s_qa, 1)
                grp(qT, Ng, [0, 1])
                tensor.wait_ge(s_a2b, 1)
                grp(A2T, Xz, [2, 3])
                tensor.wait_ge(s_qb, 1)
                grp(qT, Ng, [2, 3], stop=True, inc=s_mm[0])
                # stage-2: both t-tiles accumulate into one PSUM bank
                tensor.wait_ge(s_j, 1)
                tensor.wait_ge(t_y16, 16)
                nc.tensor.matmul(po[:], joint[:, 0:BM], ytb[:, 0:YW],
                                 start=True, stop=False)
                nc.tensor.matmul(po[:], joint[:, BM:2 * BM], ytb[:, YW:2 * YW],
                                 start=False, stop=True).then_inc(s_s2, 1)

    nc.finalize()
    return nc


_PROG = None


def _get_prog() -> bass.Bass:
    global _PROG
    if _PROG is None:
        _PROG = build_program()
    return _PROG


def _snmajor(tbl: np.ndarray) -> np.ndarray:
    """(TSH, SN) row-major -> (128, 4*TSH) sn-chunk-major bf16."""
    return np.ascontiguousarray(
        tbl.T.reshape(4, 128, TSH).transpose(1, 0, 2).reshape(128, 4 * TSH)
    ).astype(NPBF)


def make_in_maps(mean, log_var, mean_T, log_var_T, y_true_T, eps):
    f = np.float32
    mean32 = np.asarray(mean, f).reshape(B, SN)
    lv32 = np.asarray(log_var, f).reshape(B, SN)
    eps32 = np.asarray(eps, f).reshape(BM, SN)
    lvT = np.asarray(log_var_T, f).reshape(T, SN)
    mT = np.asarray(mean_T, f).reshape(T, SN)
    yT = np.asarray(y_true_T, f).reshape(T, SY)

    cval = f(KONST + (S * 0.5) * np.sum(lvT[0, :N], dtype=np.float64))
    ycv = np.full((128, 1), cval, f)
    # sn-major z inputs, m-duplicated to 64 columns (bm = m*B + b)
    lvd = np.tile(lv32.T, (1, M))                                 # (512, 64)
    mnd = np.tile(mean32.T, (1, M))
    epT = eps32.T                                                 # (512, 64)
    full = np.concatenate([lvd, mnd, epT], axis=1)                # (512, 192)
    zint = np.ascontiguousarray(
        full.reshape(4, 128, ZW).transpose(1, 0, 2).reshape(128, 4 * ZW)
    ).astype(NPBF)

    in_maps = []
    for cix in range(NCORES):
        sl = slice(cix * TSH, (cix + 1) * TSH)
        lvs, mts, ys = lvT[sl], mT[sl], yT[sl]
        ytb = np.zeros((128, 2 * YW), NPBF)
        for ti, t0 in enumerate(T_TILES):
            ytb[:, ti * YW:ti * YW + SY] = ys[t0:t0 + 128].astype(NPBF)
            ytb[:, ti * YW + SY] = NPBF(1.0)
        # the second tile overlaps the first on t 122..127: zero its y/ones
        # rows so those prototypes are counted once in stage-2
        dup = 128 - T_TILES[1]   # number of duplicated rows = 6
        ytb[0:dup, YW:2 * YW] = NPBF(0.0)
        in_maps.append({
            "lvt": _snmajor(lvs),
            "mtt": _snmajor(mts),
            "m2t": _snmajor(mts * mts),
            "zint": zint,
            "ytb": ytb,
            "ycv": ycv,
        })
    return in_maps


def finish(partials) -> np.ndarray:
    """Host epilogue: sum per-core partials, divide, mean over m, clip."""
    tot = np.sum(np.stack([np.asarray(p, np.float32).reshape(BM, SY + 1)
                           for p in partials]), axis=0, dtype=np.float32)
    num_y = tot[:, :SY].reshape(M, B, S, Y)
    num_j = tot[:, SY].reshape(M, B, 1, 1)
    probs = np.maximum(num_y, np.float32(1e-20)) / np.maximum(num_j, np.float32(1e-20))
    prob = np.sum(probs, axis=0, dtype=np.float32) / np.float32(M)
    return np.clip(prob, 0.0, 1.0).astype(np.float32)


def kernel(mean, log_var, mean_T, log_var_T, y_true_T, eps) -> np.ndarray:
    from concourse.bass_utils import run_bass_kernel_spmd

    nc = _get_prog()
    in_maps = make_in_maps(mean, log_var, mean_T, log_var_T, y_true_T, eps)
    res = run_bass_kernel_spmd(nc, in_maps, list(range(NCORES))).results
    return finish([r["partial"] for r in res])


# revision 21
# speedup vs baseline: 1.0172x; 1.0172x over previous
"""Trainium2 Bass kernel for nn_CIP_44392781971895.

Math: the reference computes, per (b, m, t),
    joint[b,m,t] = min( prod_{s,n} pdf(z[b,m,s,n]; mean_T[t,s,n], var[t,s,n])
                        * 4.13273 * std_T0[n], 1e20 )
then num_y = einsum('bmt,tsy', joint, y_true_T), num = sum_t joint,
probs = max(num_y,1e-20)/max(num,1e-20), mean over m, clip to [0,1].

The product over the 512 (s,n) pairs is computed in log space, which
collapses to a matmul over the flattened sn axis:

    logit[t,bm] = cval + sum_sn[ A2*z - 0.5*e*z^2 - 0.5*lvT - 0.5*q ]
      e  = exp(-log_var_T)   (= 1/var; the reference's 1e-20 variance
           floor binds only for log_var_T < -46, far outside the input
           distribution, so it is not applied)
      A2 = e * mean_T,  q = e * mean_T^2
      cval = S*N*(log 4.13273 - 0.5 log 2pi) + (S/2) * sum_n log_var_T[0,0,:]
    joint = exp(logit)
(The reference's min(.,1e20) clamp binds only for logit > 46; the log-joints
for this problem sit far below the fp32-exp underflow threshold, with 380+
log-units of margin, so the clamp is inert and omitted.)

All tables are laid out sn-major (pre-transposed on the host), so the
contraction runs directly over the partition axis with NO on-device
transposes: 4 chunks of 128 sn-rows, t in the free dimension.  The per-t
constants sum(-0.5*lvT) and sum(-0.5*q) are folded into the same PSUM
accumulation as extra matmul chunks against a constant -0.5 tile, and cval
enters through the exp bias column.

Sharding: the T=2000 prototype axis is split across the 8 cores (250 each).
The shard is covered by two overlapping 128-wide t-tiles (0:128, 122:250);
the 6 duplicated prototypes have their y/ones columns zeroed in the second
tile so stage-2 counts them once.  Both tiles' logits live in one PSUM bank
([128, 128]) and share a single exp; both stage-2 matmuls accumulate into
one PSUM bank, DMA'd out as a single (64, 161) partial that the host sums
across cores and finishes (divide / mean over m / clip).

Precision: tables, z samples, stage-1 operands, joints, and the stage-2
operands are bf16 (fp32 PSUM accumulation everywhere).  The bf16-induced
logit error (a few units) is inconsequential against the 380+ log-unit
underflow margin, and y_true in bf16 is well inside the 2e-2 tolerance.

Raw Bass (explicit engine blocks + single-event semaphores); input DMAs
are spread across the SP and Pool queues so they issue concurrently, an
early dummy matmul starts the PE clock-ramp, and the output copy + DMA
ride the Activation queue back-to-back.
"""

from contextlib import ExitStack

import ml_dtypes
import numpy as np

import concourse.bass as bass
import concourse.mybir as mybir

NCORES = 8
B, S, N = 32, 16, 32
T, M, Y = 2000, 2, 10
SN = S * N            # 512  (contraction length per prototype)
BM = B * M            # 64   (flattened batch*samples, column index m*B + b)
TSH = T // NCORES     # 250  (prototypes per core)
SY = S * Y            # 160
F32 = mybir.dt.float32
BF16 = mybir.dt.bfloat16
NPBF = ml_dtypes.bfloat16

KONST = float(SN * (np.log(np.float64(4.13273)) - 0.5 * np.log(2.0 * np.pi)))

T_TILES = [0, TSH - 128]   # start t of the two (overlapping) 128-wide tiles
ZW = 192                   # per-chunk zint row: lv|mean|eps
YW = SY + 1                # per-tile ytb16 row: y(160) | ones


def build_program() -> bass.Bass:
    nc = bass.Bass()
    AF = mybir.ActivationFunctionType
    OP = mybir.AluOpType

    # Packed inputs (built host-side in make_in_maps), tables sn-chunk-major:
    #   lvt:  (128, 1000) bf16  lvt[p, c*250+j] = log_var_T[shard j, sn c*128+p]
    #   mtt:  (128, 1000) bf16  same layout for mean_T
    #   m2t:  (128, 1000) bf16  same layout for mean_T^2
    #   zint: (128, 768)  bf16  chunk c cols [c*192,(c+1)*192) =
    #         [lv.T dup(64) | mean.T dup(64) | eps.T(64)] for sn c*128+p
    #   ytb:  (128, 322)  bf16  tile ti cols [ti*161,...): [y(160) | 1]
    #   ycv:  (128, 1)    f32   cval (exp bias column)
    lvt_d = nc.dram_tensor("lvt", [128, 4 * TSH], BF16, kind="ExternalInput")
    mtt_d = nc.dram_tensor("mtt", [128, 4 * TSH], BF16, kind="ExternalInput")
    m2t_d = nc.dram_tensor("m2t", [128, 4 * TSH], BF16, kind="ExternalInput")
    zint_d = nc.dram_tensor("zint", [128, 4 * ZW], BF16, kind="ExternalInput")
    ytb_d = nc.dram_tensor("ytb", [128, 2 * YW], BF16, kind="ExternalInput")
    ycv_d = nc.dram_tensor("ycv", [128, 1], F32, kind="ExternalInput")
    part_d = nc.dram_tensor("partial", [BM, SY + 1], F32, kind="ExternalOutput")

    es = ExitStack()
    with es:
        sb = lambda name, shape, dt=BF16: es.enter_context(nc.sbuf_tensor(name, shape, dt))
        ps = lambda name, shape, dt: es.enter_context(nc.psum_tensor(name, shape, dt))

        lvt = sb("s_lvt", [128, 4 * TSH])
        mtt = sb("s_mtt", [128, 4 * TSH])
        m2t = sb("s_m2t", [128, 4 * TSH])
        zint = sb("s_zint", [128, 4 * ZW])
        ytb = sb("s_ytb", [128, 2 * YW])
        ycv = sb("s_ycv", [128, 1], F32)
        eT = sb("s_eT", [128, 4 * TSH])
        A2T = sb("s_A2T", [128, 4 * TSH])
        qT = sb("s_qT", [128, 4 * TSH])
        X = sb("s_X", [128, 8 * BM])       # [z chunks 0..3 | -0.5 z^2 chunks]
        std4 = sb("s_std4", [128, 4 * BM])
        joint = sb("s_joint", [128, 2 * BM])
        neg64 = sb("s_neg64", [128, BM])   # bf16 -0.5 tile (C-chunk rhs)
        out_sb = sb("s_out", [BM, SY + 1], F32)
        bz16 = sb("s_bz16", [128, 1])      # bf16 zeros (exp bias)
        warm = sb("s_warm", [1, 1])

        pl = ps("p_l", [128, 2 * BM], F32)   # logits, tile ti in cols ti*64..
        po = ps("p_o", [BM, SY + 1], F32)
        pdum = ps("p_dum", [BM, 1], F32)

        sem = lambda name: es.enter_context(nc.semaphore(name))
        t_lv, t_mt, t_m2, t_z = (sem(n) for n in ("t_lv", "t_mt", "t_m2", "t_z"))
        t_y16, t_yc = sem("t_y16"), sem("t_yc")
        s_bias, s_ng, s_std, s_x = sem("s_bias"), sem("s_ng"), sem("s_std"), sem("s_x")
        s_ea, s_eb = sem("s_ea"), sem("s_eb")
        s_a2a, s_a2b = sem("s_a2a"), sem("s_a2b")
        s_qa, s_qb = sem("s_qa"), sem("s_qb")
        s_mm = [sem("s_mm0"), sem("s_mm1")]
        s_j, s_s2, s_oc, s_od = sem("s_j"), sem("s_s2"), sem("s_oc"), sem("s_od")
        s_oc2 = sem("s_oc2")

        zview = zint[:].rearrange("p (c k) -> p c k", k=ZW)
        lv4 = zview[:, :, 0:BM]
        mean4 = zview[:, :, BM:2 * BM]
        eps4 = zview[:, :, 2 * BM:3 * BM]
        std4v = std4[:].rearrange("p (c k) -> p c k", k=BM)
        X0v = X[:, 0:4 * BM].rearrange("p (c k) -> p c k", k=BM)

        def tsl(tbl, c, ti):
            t0 = T_TILES[ti]
            return tbl[:, c * TSH + t0: c * TSH + t0 + 128]

        H = 2 * TSH   # column split of the sn-major tables (chunks 0-1 | 2-3)

        with nc.Block() as block:

            @block.sync
            def _(sync):
                sync.dma_start(zint[:], zint_d[:]).then_inc(t_z, 16)
                sync.dma_start(lvt[:], lvt_d[:]).then_inc(t_lv, 16)
                sync.dma_start(mtt[:], mtt_d[:]).then_inc(t_mt, 16)
                sync.dma_start(ycv[:], ycv_d[:]).then_inc(t_yc, 16)

            @block.gpsimd
            def _(gp):
                gp.memset(bz16[:], 0.0).then_inc(s_bias, 1)
                gp.dma_start(m2t[:], m2t_d[:]).then_inc(t_m2, 16)
                gp.dma_start(ytb[:], ytb_d[:]).then_inc(t_y16, 16)
                gp.wait_ge(s_ea, 1)
                gp.wait_ge(t_mt, 16)
                gp.tensor_mul(A2T[:, 0:H], eT[:, 0:H], mtt[:, 0:H]).then_inc(s_a2a, 1)
                gp.wait_ge(t_m2, 16)
                gp.tensor_mul(qT[:, 0:H], eT[:, 0:H], m2t[:, 0:H]).then_inc(s_qa, 1)
                gp.wait_ge(s_eb, 1)
                gp.tensor_mul(A2T[:, H:2 * H], eT[:, H:2 * H],
                              mtt[:, H:2 * H]).then_inc(s_a2b, 1)

            @block.vector
            def _(vector):
                vector.memset(neg64[:], -0.5).then_inc(s_ng, 1)
                vector.wait_ge(t_z, 16)
                vector.wait_ge(s_std, 1)
                vector.tensor_mul(X0v, eps4, std4v)
                vector.drain()
                vector.tensor_add(X0v, X0v, mean4)
                vector.drain()
                vector.scalar_tensor_tensor(
                    X[:, 4 * BM:8 * BM], X[:, 0:4 * BM], -0.5, X[:, 0:4 * BM],
                    op0=OP.mult, op1=OP.mult).then_inc(s_x, 1)
                vector.wait_ge(s_eb, 1)
                vector.wait_ge(t_m2, 16)
                vector.tensor_mul(qT[:, H:2 * H], eT[:, H:2 * H],
                                  m2t[:, H:2 * H]).then_inc(s_qb, 1)
                vector.wait_ge(s_s2, 1)
                vector.tensor_copy(out_sb[:, 80:SY + 1],
                                   po[:, 80:SY + 1]).then_inc(s_oc2, 1)

            @block.scalar
            def _(scalar):
                # prewarm the ACT Exp table while DMAs are in flight
                scalar.wait_ge(s_bias, 1)
                scalar.activation(warm[:], bz16[0:1, :], AF.Exp,
                                  bias=bz16[0:1, :])
                scalar.wait_ge(t_z, 16)
                scalar.activation(std4[:], lv4, AF.Exp, bias=bz16[:, :],
                                  scale=0.5).then_inc(s_std, 1)
                scalar.wait_ge(t_lv, 16)
                scalar.activation(eT[:, 0:H], lvt[:, 0:H], AF.Exp,
                                  bias=bz16[:, :], scale=-1.0).then_inc(s_ea, 1)
                scalar.activation(eT[:, H:2 * H], lvt[:, H:2 * H], AF.Exp,
                                  bias=bz16[:, :], scale=-1.0).then_inc(s_eb, 1)
                # joints: one exp over both tiles' logits, cval via bias
                scalar.wait_ge(s_mm[0], 1)
                scalar.wait_ge(t_yc, 16)
                scalar.activation(joint[:], pl[:, :], AF.Exp,
                                  bias=ycv[:, 0:1]).then_inc(s_j, 1)
                # output: PSUM -> SBUF (split with DVE) -> DRAM on this queue
                scalar.wait_ge(s_s2, 1)
                scalar.copy(out_sb[:, 0:80], po[:, 0:80]).then_inc(s_oc, 1)
                scalar.wait_ge(s_oc, 1)
                scalar.wait_ge(s_oc2, 1)
                scalar.dma_start(part_d[:], out_sb[:]).then_inc(s_od, 16)

            @block.tensor
            def _(tensor):
                # dummy matmul: start the PE p-state ramp clock early
                tensor.wait_ge(s_ng, 1)
                nc.tensor.matmul(pdum[:], neg64[:, 0:BM], neg64[:, 0:1],
                                 start=True, stop=True)
                # stage-1: 16 chunk-matmuls per 128-wide t-tile accumulating
                #   logit = z@A2 - 0.5 z^2 @ e - 0.5*(lvT + q) @ 1
                # into pl cols ti*64..; ordered by operand readiness.
                def grp(tbl, rhs_of, cs, start=False, stop=False, inc=None):
                    for ti in range(2):
                        for c in cs:
                            ins = nc.tensor.matmul(
                                pl[:, ti * BM:(ti + 1) * BM], tsl(tbl, c, ti),
                                rhs_of(c),
                                start=start and ti == 0 and c == cs[0],
                                stop=stop and ti == 1 and c == cs[-1])
                            if inc is not None and ti == 1 and c == cs[-1]:
                                ins.then_inc(inc, 1)

                Xz = lambda c: X[:, c * BM:(c + 1) * BM]
                X2 = lambda c: X[:, (4 + c) * BM:(5 + c) * BM]
                Ng = lambda c: neg64[:, 0:BM]

                tensor.wait_ge(t_lv, 16)
                grp(lvt, Ng, [0, 1, 2, 3], start=True)
                tensor.wait_ge(s_ea, 1)
                tensor.wait_ge(s_x, 1)
                grp(eT, X2, [0, 1])
                tensor.wait_ge(s_eb, 1)
                grp(eT, X2, [2, 3])
                tensor.wait_ge(s_a2a, 1)
                grp(A2T, Xz, [0, 1])
                tensor.wait_ge(s_qa, 1)
                grp(qT, Ng, [0, 1])
                tensor.wait_ge(s_qb, 1)
                grp(qT, Ng, [2, 3])
                tensor.wait_ge(s_a2b, 1)
                grp(A2T, Xz, [2, 3], stop=True, inc=s_mm[0])
                # stage-2: both t-tiles accumulate into one PSUM bank
                tensor.wait_ge(s_j, 1)
                tensor.wait_ge(t_y16, 16)
                nc.tensor.matmul(po[:], joint[:, 0:BM], ytb[:, 0:YW],
                                 start=True, stop=False)
                nc.tensor.matmul(po[:], joint[:, BM:2 * BM], ytb[:, YW:2 * YW],
                                 start=False, stop=True).then_inc(s_s2, 1)

    nc.finalize()
    return nc


_PROG = None


def _get_prog() -> bass.Bass:
    global _PROG
    if _PROG is None:
        _PROG = build_program()
    return _PROG


def _snmajor(tbl: np.ndarray) -> np.ndarray:
    """(TSH, SN) row-major -> (128, 4*TSH) sn-chunk-major bf16."""
    return np.ascontiguousarray(
        tbl.T.reshape(4, 128, TSH).transpose(1, 0, 2).reshape(128, 4 * TSH)
    ).astype(NPBF)


def make_in_maps(mean, log_var, mean_T, log_var_T, y_true_T, eps):
    f = np.float32
    mean32 = np.asarray(mean, f).reshape(B, SN)
    lv32 = np.asarray(log_var, f).reshape(B, SN)
    eps32 = np.asarray(eps, f).reshape(BM, SN)
    lvT = np.asarray(log_var_T, f).reshape(T, SN)
    mT = np.asarray(mean_T, f).reshape(T, SN)
    yT = np.asarray(y_true_T, f).reshape(T, SY)

    cval = f(KONST + (S * 0.5) * np.sum(lvT[0, :N], dtype=np.float64))
    ycv = np.full((128, 1), cval, f)
    # sn-major z inputs, m-duplicated to 64 columns (bm = m*B + b)
    lvd = np.tile(lv32.T, (1, M))                                 # (512, 64)
    mnd = np.tile(mean32.T, (1, M))
    epT = eps32.T                                                 # (512, 64)
    full = np.concatenate([lvd, mnd, epT], axis=1)                # (512, 192)
    zint = np.ascontiguousarray(
        full.reshape(4, 128, ZW).transpose(1, 0, 2).reshape(128, 4 * ZW)
    ).astype(NPBF)

    in_maps = []
    for cix in range(NCORES):
        sl = slice(cix * TSH, (cix + 1) * TSH)
        lvs, mts, ys = lvT[sl], mT[sl], yT[sl]
        ytb = np.zeros((128, 2 * YW), NPBF)
        for ti, t0 in enumerate(T_TILES):
            ytb[:, ti * YW:ti * YW + SY] = ys[t0:t0 + 128].astype(NPBF)
            ytb[:, ti * YW + SY] = NPBF(1.0)
        # the second tile overlaps the first on t 122..127: zero its y/ones
        # rows so those prototypes are counted once in stage-2
        dup = 128 - T_TILES[1]   # number of duplicated rows = 6
        ytb[0:dup, YW:2 * YW] = NPBF(0.0)
        in_maps.append({
            "lvt": _snmajor(lvs),
            "mtt": _snmajor(mts),
            "m2t": _snmajor(mts * mts),
            "zint": zint,
            "ytb": ytb,
            "ycv": ycv,
        })
    return in_maps


def finish(partials) -> np.ndarray:
    """Host epilogue: sum per-core partials, divide, mean over m, clip."""
    tot = np.sum(np.stack([np.asarray(p, np.float32).reshape(BM, SY + 1)
                           for p in partials]), axis=0, dtype=np.float32)
    num_y = tot[:, :SY].reshape(M, B, S, Y)
    num_j = tot[:, SY].reshape(M, B, 1, 1)
    probs = np.maximum(num_y, np.float32(1e-20)) / np.maximum(num_j, np.float32(1e-20))
    prob = np.sum(probs, axis=0, dtype=np.float32) / np.float32(M)
    return np.clip(prob, 0.0, 1.0).astype(np.float32)


def kernel(mean, log_var, mean_T, log_var_T, y_true_T, eps) -> np.ndarray:
    from concourse.bass_utils import run_bass_kernel_spmd

    nc = _get_prog()
    in_maps = make_in_maps(mean, log_var, mean_T, log_var_T, y_true_T, eps)
    res = run_bass_kernel_spmd(nc, in_maps, list(range(NCORES))).results
    return finish([r["partial"] for r in res])
